# revision 23
# baseline (speedup 1.0000x reference)
"""GCN (5-layer PyG GCNConv + BatchNorm eval + ReLU) on 8 Trainium2 NeuronCores.

v3 design. Nodes dst-sharded across 8 cores (12544 padded rows, 98 tiles of
128); edges follow their destination. The five layers run as one rolling
group-level software pipeline: for each group of dst tiles, edge-source rows
are gathered (dma_gather, 4 SWDGE queues round-robin), per-tile one-hot
selection matrices are built on DVE, aggregation runs as PSUM-accumulating
matmuls, the epilogue (fused BN+ReLU) feeds the NEXT layer's H-phase matmuls
for the same tiles, and chunked AllGathers of the next layer's node features
are triggered at quarter boundaries so collectives overlap the gather drain.

Gather descriptors are the bottleneck (~7.5ns/descriptor/queue), so edges are
packed into pooled per-(group,quarter) blocks (tile-sorted, boundary blocks
mix two dst tiles and get one masked one-hot matmul per tile) instead of
per-tile rounding: ~211k descriptors/layer vs 251k. Layers 4 and 5 use 64-wide
tables gathered as 2-node pairs (256B elements) with parity-split one-hots,
and layer 5 aggregates h4_act directly, applying W5 after aggregation (the
"H phase" of layer 5 is a single fused scale inside layer 4's epilogue).

GCN_REPS=n unrolls the computation n times on-device so test.py can measure
execution time as a wall-clock slope independent of tunnel latency.
"""
import os
import numpy as np
import ml_dtypes

N = 100000
E = 1600000
IN = 128
H = 128
C = 2
EPS = 1e-5
NC = 8
SR = 12500
P = 128
TP = 98
SH = TP * P
NQ = 4
QT = [25, 25, 24, 24]
QTOFF = [0, 25, 50, 74]
QROFF = [0, 3200, 6400, 9472]
QROWS = [3200, 3200, 3072, 3072]
GROUPS = [5] * 10 + [4] * 12
NG = len(GROUPS)
GOFF = np.cumsum([0] + GROUPS).tolist()
QG = [0] * 5 + [1] * 5 + [2] * 6 + [3] * 6
QLASTG = [4, 9, 15, 21]
QFIRSTG = [0, 5, 10, 16]
OC = [128, 128, 128, 64, 64]   # gather-table width per layer
DIMS = [(IN, H), (H, H), (H, H), (H, H // 2), (H // 2, C)]

_cache = {}

# ---------------------------------------------------------------------------
# Tile patch: walrus in this container rejects TPB_CTRL/extended instructions
# with >1 sync wait. Split waits across single-wait NOPs.
# ---------------------------------------------------------------------------


def _apply_tile_patch():
    if _cache.get("patched"):
        return
    _cache["patched"] = True
    import concourse.tile as tile_mod
    import concourse.mybir as mybir
    from concourse.vector_clock import ScopedClock

    MAXW = 1

    def _patched_drain_and_barrier(self, tick_clock, wait_clock):
        nc = self.nc
        probe = nc.sync.nop(nofuse=True)
        wait_clock.add_sem_waits(probe.ins, ScopedClock({None: tick_clock.global_clock}))
        si = probe.ins.sync_info
        if si is not None and si.on_wait and len(si.on_wait) > MAXW:
            waits = list(si.on_wait)
            si.on_wait = waits[:MAXW]
            for k in range(MAXW, len(waits), MAXW):
                extra = nc.sync.nop(nofuse=True)
                esi = extra.ins.sync_info
                if esi is None:
                    extra.ins.sync_info = mybir.SyncInfo(
                        on_wait=waits[k:k + MAXW], on_update=[]
                    )
                else:
                    esi.on_wait = waits[k:k + MAXW]
        nc.sync.drain()
        nc.all_engine_barrier()
        assert self.sems is not None
        popped = nc._tile_sem_poison_stack.pop()
        assert popped is self._sem_poison
        nc.clear_and_free_semaphores(list(self.sems.allocated().values()))
        nc.all_engine_barrier()

    tile_mod.TileContext._drain_and_barrier = _patched_drain_and_barrier

    _orig_commit = tile_mod.TileContext._commit_instruction

    def _patched_commit_instruction(self, inst, lazy_reg_writes=True):
        si = getattr(inst, "sync_info", None)
        if (
            si is not None
            and si.on_wait
            and len(si.on_wait) > MAXW
            and inst.engine != mybir.EngineType.Unassigned
        ):
            waits = list(si.on_wait)
            si.on_wait = waits[:MAXW]
            eng = self.nc.engines[inst.engine]
            for k in range(MAXW, len(waits), MAXW):
                extra = eng.nop(nofuse=True)
                esi = extra.ins.sync_info
                chunk = waits[k:k + MAXW]
                if esi is None:
                    extra.ins.sync_info = mybir.SyncInfo(on_wait=chunk, on_update=[])
                else:
                    esi.on_wait = chunk
        return _orig_commit(self, inst, lazy_reg_writes)

    tile_mod.TileContext._commit_instruction = _patched_commit_instruction


# ---------------------------------------------------------------------------
# SPMD runner: compile once via bass2jax/PJRT, keep the jitted fn for reuse.
# ---------------------------------------------------------------------------


class _SpmdRunner:
    def __init__(self, nc, n_cores=8):
        import jax
        from jax.sharding import Mesh, PartitionSpec, NamedSharding
        from jax.experimental.shard_map import shard_map
        import concourse.mybir as mybir
        from concourse.bass2jax import (
            _bass_exec_p,
            install_neuronx_cc_hook,
            partition_id_tensor,
        )
        from concourse.library_overlay import lower_extended_insts

        lower_extended_insts(nc)
        install_neuronx_cc_hook()
        self.jax = jax
        self.n_cores = n_cores
        partition_name = nc.partition_id_tensor.name if nc.partition_id_tensor else None
        in_names, out_names, out_avals, zero_outs = [], [], [], []
        for alloc in nc.m.functions[0].allocations:
            if not isinstance(alloc, mybir.MemoryLocationSet):
                continue
            name = alloc.memorylocations[0].name
            if alloc.kind == "ExternalInput":
                if name != partition_name:
                    in_names.append(name)
            elif alloc.kind == "ExternalOutput":
                out_names.append(name)
                shape = tuple(alloc.tensor_shape)
                dtype = mybir.dt.np(alloc.dtype)
                out_avals.append(jax.core.ShapedArray(shape, dtype))
                zero_outs.append(np.zeros(shape, dtype))
        self.in_names = list(in_names)
        self.out_names = out_names
        self.out_avals = out_avals
        self.zero_outs = zero_outs
        n_params = len(in_names)
        n_outs = len(out_avals)
        all_in_names = list(in_names) + list(out_names)
        if partition_name is not None:
            all_in_names.append(partition_name)

        def _body(*args):
            operands = list(args)
            if partition_name is not None:
                operands.append(partition_id_tensor())
            outs = _bass_exec_p.bind(
                *operands,
                out_avals=tuple(out_avals),
                in_names=tuple(all_in_names),
                out_names=tuple(out_names),
                lowering_input_output_aliases=(),
                sim_require_finite=True,
                sim_require_nnan=True,
                nc=nc,
            )
            return tuple(outs)

        devices = jax.devices()[:n_cores]
        self.mesh = Mesh(np.asarray(devices), ("core",))
        in_specs = (PartitionSpec("core"),) * (n_params + n_outs)
        out_specs = (PartitionSpec("core"),) * n_outs
        self.sharding = NamedSharding(self.mesh, PartitionSpec("core"))
        self.fn = jax.jit(
            shard_map(
                _body, mesh=self.mesh, in_specs=in_specs, out_specs=out_specs,
                check_rep=False,
            ),
            keep_unused=True,
        )
        self.n_params = n_params

    def put_inputs(self, in_maps):
        jax = self.jax
        per_core = [[np.asarray(m[name]) for name in self.in_names] for m in in_maps]
        concat_in = [
            np.concatenate([per_core[c][i] for c in range(self.n_cores)], axis=0)
            for i in range(self.n_params)
        ]
        self.dev_in = [jax.device_put(a, self.sharding) for a in concat_in]
        self.dev_zeros = [
            jax.device_put(
                np.zeros((self.n_cores * z.shape[0], *z.shape[1:]), z.dtype),
                self.sharding,
            )
            for z in self.zero_outs
        ]
        jax.block_until_ready(self.dev_in)

    def run(self):
        outs = self.fn(*self.dev_in, *self.dev_zeros)
        self.jax.block_until_ready(outs)
        return outs

    def results(self, outs):
        res = []
        for c in range(self.n_cores):
            res.append(
                {
                    name: np.asarray(outs[i]).reshape(
                        self.n_cores, *self.out_avals[i].shape
                    )[c]
                    for i, name in enumerate(self.out_names)
                }
            )
        return res


# ---------------------------------------------------------------------------
# Host-side graph partitioning
# ---------------------------------------------------------------------------


def _host_prep(edge_index):
    src = np.asarray(edge_index[0], dtype=np.int64)
    dst = np.asarray(edge_index[1], dtype=np.int64)
    deg = np.bincount(dst, minlength=N).astype(np.float32) + 1.0
    dinv = (1.0 / np.sqrt(deg)).astype(np.float32)

    core = dst // SR
    dl = dst - core * SR
    tile = dl // P
    slot = dl % P
    grp = np.searchsorted(np.asarray(GOFF[1:]), tile, side="right")

    cs = src // SR
    r = src - cs * SR
    q = (r >= QROFF[1]).astype(np.int64) + (r >= QROFF[2]) + (r >= QROFF[3])
    qrows = np.asarray(QROWS)[q]
    qidx = cs * qrows + (r - np.asarray(QROFF)[q])

    cell = (core * NG + grp) * NQ + q
    order = np.lexsort((qidx, tile, cell))
    cell_s = cell[order]
    tile_s = tile[order]
    slot_s = slot[order]
    qidx_s = qidx[order]

    ncells = NC * NG * NQ
    cnt = np.bincount(cell_s, minlength=ncells).reshape(NC, NG, NQ)
    blocks = np.maximum(np.ceil(cnt.max(axis=0) / P).astype(np.int64), 1)

    cumq = np.zeros((NG, NQ + 1), np.int64)
    for g in range(NG):
        cumq[g, 1:] = np.cumsum(blocks[g])
    gw = cumq[:, -1]
    goffc = np.concatenate([[0], np.cumsum(gw)])
    DW = int(goffc[-1])

    cstart = np.zeros(ncells + 1, np.int64)
    np.cumsum(cnt.reshape(-1), out=cstart[1:])
    rank = np.arange(E) - cstart[cell_s]
    g_s = (cell_s // NQ) % NG
    q_s = cell_s % NQ
    c_s = cell_s // (NG * NQ)
    colrel = rank // P
    rowrel = rank % P
    gcol = goffc[g_s] + cumq[g_s, q_s] + colrel

    # pieces keyed/sorted by (group, tile, quarter, block) so each tile's
    # dsel columns are contiguous
    KQ = int(blocks.max()) + 1
    key = ((tile_s * NQ + q_s) * KQ) + colrel
    uk = np.unique(key)
    NPIECE = len(uk)
    pcol = np.searchsorted(uk, key)
    piece_tile = uk // (NQ * KQ)
    piece_q = (uk // KQ) % NQ
    piece_lc = uk % KQ  # block within (g, q)

    dsel = np.full((NC, P, NPIECE), -1.0, np.float32)
    dsel[c_s, rowrel, pcol] = slot_s.astype(np.float32)
    dsel_par = np.full((NC, P, 2 * NPIECE), -1.0, np.float32)
    dsel_par[c_s, rowrel, 2 * pcol + (qidx_s & 1)] = slot_s.astype(np.float32)

    idx_row = np.zeros((NC, DW, P), np.int16)
    idx_row[c_s, gcol, rowrel] = qidx_s.astype(np.int16)
    idx_pair = np.zeros((NC, DW, P), np.int16)
    idx_pair[c_s, gcol, rowrel] = (qidx_s >> 1).astype(np.int16)

    def wrap(idx_flat):
        iw = np.zeros((NC, P, DW * 8), np.int16)
        for g in range(NG):
            for qq in range(NQ):
                c0 = int(goffc[g] + cumq[g, qq])
                w = int(blocks[g, qq])
                seq = idx_flat[:, c0:c0 + w, :].reshape(NC, w * P)
                w16 = seq.reshape(NC, w * P // 16, 16).transpose(0, 2, 1)
                iw[:, :, c0 * 8:(c0 + w) * 8] = np.tile(w16, (1, 8, 1))
        return iw

    idx_row_w = wrap(idx_row)
    idx_pair_w = wrap(idx_pair)

    # per-(group, tile) piece lists: [(q, local block col in group, piece col)]
    tg = np.searchsorted(np.asarray(GOFF[1:]), piece_tile, side="right")
    plan = [dict() for _ in range(NG)]
    for i in range(NPIECE):
        g = int(tg[i])
        t = int(piece_tile[i])
        lc = int(cumq[g, piece_q[i]] + piece_lc[i])
        plan[g].setdefault(t, []).append((int(piece_q[i]), lc, i))

    meta = dict(blocks=blocks.tolist(), cumq=cumq.tolist(),
                gw=gw.tolist(), goffc=goffc.tolist(), DW=DW,
                NPIECE=NPIECE, plan=plan)
    return dinv, idx_row_w, idx_pair_w, dsel, dsel_par, meta


def _fold_weights(inputs):
    Ws, Bs = [], []
    for i in range(1, 6):
        W = np.asarray(inputs[f"W{i}"], np.float32)
        b = np.asarray(inputs[f"b{i}"], np.float32)
        if i <= 4:
            g = np.asarray(inputs[f"g{i}"], np.float32)
            be = np.asarray(inputs[f"be{i}"], np.float32)
            rm = np.asarray(inputs[f"rm{i}"], np.float32)
            rv = np.asarray(inputs[f"rv{i}"], np.float32)
            s = g / np.sqrt(rv + EPS)
            W = W * s[None, :]
            b = b * s + be - rm * s
        Ws.append(np.ascontiguousarray(W, dtype=np.float32))
        Bs.append(b.astype(np.float32)[None, :])
    return Ws, Bs


# ---------------------------------------------------------------------------
# Device program
# ---------------------------------------------------------------------------


def _build_nc(meta):
    REPS = int(os.environ.get("GCN_REPS", 1))
    NQUEUES = int(os.environ.get("GCN_QUEUES", 4))
    import concourse.bass as bass
    import concourse.mybir as mybir
    from concourse.tile import TileContext
    from concourse import library_config

    _apply_tile_patch()

    f32 = mybir.dt.float32
    bf16 = mybir.dt.bfloat16
    i16 = mybir.dt.int16
    nc = bass.Bass(
        "TRN2", target_bir_lowering=False, debug=False, num_swdge_queues=NQUEUES
    )

    blocks = meta["blocks"]      # [NG][NQ]
    cumq = meta["cumq"]          # [NG][NQ+1]
    gw = meta["gw"]              # [NG]
    goffc = meta["goffc"]        # [NG+1]
    DW = meta["DW"]
    NPIECE = meta["NPIECE"]
    plan = meta["plan"]          # [NG] {tile: [(q, lc, piece)]}
    GWMAX = max(gw)
    NPT_MAX = max(len(v) for g in range(NG) for v in plan[g].values())
    BMAX = max(max(b) for b in blocks)

    xT_in = nc.declare_dram_parameter("xT", [IN, SH], bf16, isOutput=False)
    dinv_in = nc.declare_dram_parameter("dinv", [P, TP], f32, isOutput=False)
    dinv2_in = nc.declare_dram_parameter("dinv2", [P, TP], f32, isOutput=False)
    rdinv_in = nc.declare_dram_parameter("rdinv", [1, SH], bf16, isOutput=False)
    idxr_in = nc.declare_dram_parameter("idxr", [P, DW * 8], i16, isOutput=False)
    idxp_in = nc.declare_dram_parameter("idxp", [P, DW * 8], i16, isOutput=False)
    dsel_in = nc.declare_dram_parameter("dsel", [P, NPIECE], bf16, isOutput=False)
    dselp_in = nc.declare_dram_parameter("dselp", [P, 2 * NPIECE], bf16, isOutput=False)
    iota_in = nc.declare_dram_parameter("iotaw", [P, P], bf16, isOutput=False)
    ident_in = nc.declare_dram_parameter("ident", [P, P], bf16, isOutput=False)
    W_in = [nc.declare_dram_parameter(f"W{i+1}", list(DIMS[i]), bf16, isOutput=False)
            for i in range(5)]
    B_in = [nc.declare_dram_parameter(f"B{i+1}", [1, DIMS[i][1]], bf16, isOutput=False)
            for i in range(5)]
    y_out = nc.declare_dram_parameter("y", [SH, C], f32, isOutput=True)

    in_b = [
        [nc.dram_tensor(f"in_b{l}_{q}", [QROWS[q], OC[l]], bf16) for q in range(NQ)]
        for l in range(5)
    ]
    hs = []
    for l in range(5):
        row = []
        for q in range(NQ):
            if OC[l] == 128:
                shp = [NC * QROWS[q], 128]
            else:
                shp = [NC * QROWS[q] // 2, 128]
            row.append(nc.dram_tensor(f"hs{l}_{q}", shp, bf16, addr_space="Shared"))
        hs.append(row)

    with TileContext(nc) as tc:
        with (
            tc.tile_pool(name="const", bufs=1) as cpool,
            tc.tile_pool(name="act", bufs=1) as apool,
            tc.tile_pool(name="hbuf", bufs=1) as hpool,
            tc.tile_pool(name="gath", bufs=int(os.environ.get("GCN_GBUFS", 2))) as gpool,
            tc.tile_pool(name="sel", bufs=3) as sbpool,
            tc.tile_pool(name="idxs", bufs=8) as ipool,
            tc.tile_pool(name="work", bufs=4) as wpool,
            tc.tile_pool(name="epi", bufs=8) as epool,
            tc.tile_pool(name="qp", bufs=8) as qpool,
            tc.tile_pool(name="ps_h", bufs=2, space="PSUM") as ps_h,
            tc.tile_pool(name="ps_a", bufs=2, space="PSUM") as ps_a,
            tc.tile_pool(name="ps_t", bufs=2, space="PSUM") as ps_t,
            tc.tile_pool(name="ps_y", bufs=2, space="PSUM") as ps_y,
        ):
            nc.gpsimd.load_library(library_config.mlp)
            nid_vals = sorted({blocks[g][q] * P for g in range(NG) for q in range(NQ)})
            nid_regs = {}
            for v in nid_vals:
                reg = nc.alloc_register(mybir.EngineType.Pool, f"nid{v}")
                nc.gpsimd.reg_mov(reg, v)
                nid_regs[v] = reg

            Wt, Bt = [], []
            for l in range(5):
                w = cpool.tile(list(DIMS[l]), bf16, name=f"Wt{l}")
                nc.sync.dma_start(out=w[:], in_=W_in[l][:])
                Wt.append(w)
                b = cpool.tile([1, DIMS[l][1]], bf16, name=f"Bt{l}")
                nc.sync.dma_start(out=b[:], in_=B_in[l][:])
                Bt.append(b)
            iota_t = cpool.tile([P, P], bf16)
            nc.sync.dma_start(out=iota_t[:], in_=iota_in[:])
            ident_t = cpool.tile([P, P], bf16)
            nc.sync.dma_start(out=ident_t[:], in_=ident_in[:])
            dinv_t = cpool.tile([P, TP], f32)
            nc.sync.dma_start(out=dinv_t[:], in_=dinv_in[:])
            dinv2_t = cpool.tile([P, TP], f32)
            nc.sync.dma_start(out=dinv2_t[:], in_=dinv2_in[:])
            rdinv_t = cpool.tile([1, SH], bf16)
            nc.sync.dma_start(out=rdinv_t[:], in_=rdinv_in[:])
            dsel_t = cpool.tile([P, NPIECE], bf16)
            nc.scalar.dma_start(out=dsel_t[:], in_=dsel_in[:])
            dselp_t = cpool.tile([P, 2 * NPIECE], bf16)
            nc.scalar.dma_start(out=dselp_t[:], in_=dselp_in[:])

            idxr_t = cpool.tile([P, DW * 8], i16)
            nc.scalar.dma_start(out=idxr_t[:], in_=idxr_in[:])

            qcount = [0]
            AGQ_AT = {11: 0, 16: 1, 20: 2}
            pending = []

            def flush():
                for fn in pending:
                    fn()
                pending.clear()

            def emit_H(l, g, actT, h_s):
                # layer l's H phase for group g's tiles (l = 0..3)
                I, O = DIMS[l]
                for t in range(GOFF[g], GOFF[g + 1]):
                    q = QG[g]
                    ps = ps_h.tile([P, O], f32, tag="ps_h")
                    nc.tensor.matmul(
                        out=ps[:], lhsT=actT[:I, t * P:(t + 1) * P],
                        rhs=Wt[l][:], start=True, stop=True,
                    )
                    nc.vector.tensor_scalar_mul(
                        out=h_s[:, t * O:(t + 1) * O], in0=ps[:],
                        scalar1=dinv_t[:, t:t + 1],
                    )
                    r0 = t * P - QROFF[q]
                    nc.sync.dma_start(
                        out=in_b[l][q].ap()[r0:r0 + P, :],
                        in_=h_s[:, t * O:(t + 1) * O],
                    )

            def emit_AG(l, q):
                if os.environ.get("GCN_NOCC"):
                    nrows = QROWS[q] if OC[l] == 128 else QROWS[q] // 2
                    nc.sync.dma_start(
                        out=hs[l][q].ap()[:nrows, :], in_=in_b[l][q][:]
                    )
                    return
                nc.gpsimd.collective_compute(
                    "AllGather",
                    mybir.AluOpType.bypass,
                    ins=[in_b[l][q][:]],
                    outs=[hs[l][q][:]],
                    replica_groups=[list(range(NC))],
                )

            def flush_epi(l, g, defer, actT, h_s):
                # deferred epilogue (PE transpose + DVE copy) and next-layer H
                for t, act_t in defer:
                    pt = ps_t.tile([P, P], bf16, tag="pt")
                    nc.tensor.transpose(
                        out=pt[:128, :], in_=act_t[:], identity=ident_t[:]
                    )
                    nc.vector.tensor_copy(
                        out=actT[:128, t * P:(t + 1) * P], in_=pt[:128, :]
                    )
                emit_H(l + 1, g, actT, h_s)

            def flush_y(defer):
                # deferred layer-5 output: W5 matmul + bias + dinv scale + DMA
                for t, qT in defer:
                    yp = ps_y.tile([P, C], f32, tag="yp")
                    nc.tensor.matmul(
                        out=yp[:], lhsT=qT[:], rhs=Wt[4][:],
                        start=True, stop=False,
                    )
                    nc.tensor.matmul(
                        out=yp[:], lhsT=rdinv_t[:, t * P:(t + 1) * P],
                        rhs=Bt[4][:], start=False, stop=True,
                    )
                    yt = wpool.tile([P, C], f32, tag="yt")
                    nc.vector.tensor_scalar_mul(
                        out=yt[:], in0=yp[:], scalar1=dinv_t[:, t:t + 1]
                    )
                    nc.sync.dma_start(
                        out=y_out.ap()[t * P:(t + 1) * P, :], in_=yt[:]
                    )

            for rep in range(REPS):
                actT = apool.tile([P, SH], bf16, tag="actT")
                nc.sync.dma_start(out=actT[:IN, :], in_=xT_in[:])
                h_s = hpool.tile([P, TP * P], bf16, tag="h_s")

                # prologue: layer 0 H phase + AllGathers
                for g in range(NG):
                    emit_H(0, g, actT, h_s)
                    if g in QLASTG:
                        emit_AG(0, QG[g])

                for l in range(int(os.environ.get("GCN_LAYERS", 5))):
                    O = OC[l]          # gather table width
                    pair = O == 64
                    AD = 64 if l == 4 else DIMS[l][1]   # agg feature width

                    for g in range(NG):
                        flush()
                        # --- gathers for (g, q) ---
                        gbuf = gpool.tile([P, GWMAX, P], bf16, tag="g")
                        for q in range(NQ):
                            w = blocks[g][q]
                            c0 = goffc[g] + cumq[g][q]
                            lc0 = cumq[g][q]
                            if os.environ.get("GCN_NOGATHER"):
                                nc.sync.dma_start(
                                    out=gbuf[:, lc0:lc0 + w, :],
                                    in_=hs[l][q].ap()[:w * P, :],
                                )
                                qcount[0] += 1
                                continue
                            if pair:
                                it = ipool.tile([P, BMAX * 8], i16, tag="idx")
                                nc.scalar.dma_start(
                                    out=it[:, :w * 8],
                                    in_=idxp_in.ap()[:, c0 * 8:(c0 + w) * 8],
                                )
                                idx_ap = it[:, :w * 8]
                            else:
                                idx_ap = idxr_t[:, c0 * 8:(c0 + w) * 8]
                            nc.gpsimd.dma_gather(
                                out_ap=gbuf[:, lc0:lc0 + w, :],
                                in_ap=hs[l][q].ap()[:, :],
                                idxs_ap=idx_ap,
                                num_idxs=w * P,
                                num_idxs_reg=nid_regs[w * P],
                                elem_size=P,
                                single_packet=bool(os.environ.get("GCN_SP")),
                                queue_num=qcount[0] % NQUEUES,
                            )
                            qcount[0] += 1
                        # --- per-tile: S build, agg matmuls, epilogue ---
                        defer = []
                        for t in range(GOFF[g], GOFF[g + 1]):
                            pieces = plan[g].get(t, [])
                            if os.environ.get("GCN_MMSKIP"):
                                pieces = []
                            np_t = len(pieces)
                            q = QG[g]
                            r0 = t * P - QROFF[q]
                            if np_t:
                                pc0 = pieces[0][2]
                                St = sbpool.tile([P, 2 * NPT_MAX * P], bf16, tag="S")
                                if os.environ.get("GCN_NOSBUILD"):
                                    pass
                                elif pair:
                                    nc.vector.tensor_tensor(
                                        out=St[:, :2 * np_t * P].rearrange(
                                            "p (b q) -> p b q", b=2 * np_t
                                        ),
                                        in0=iota_t[:].unsqueeze(1).broadcast_to(
                                            [P, 2 * np_t, P]
                                        ),
                                        in1=dselp_t[
                                            :, 2 * pc0:2 * (pc0 + np_t)
                                        ].unsqueeze(2).broadcast_to([P, 2 * np_t, P]),
                                        op=mybir.AluOpType.is_equal,
                                    )
                                else:
                                    nc.vector.tensor_tensor(
                                        out=St[:, :np_t * P].rearrange(
                                            "p (b q) -> p b q", b=np_t
                                        ),
                                        in0=iota_t[:].unsqueeze(1).broadcast_to(
                                            [P, np_t, P]
                                        ),
                                        in1=dsel_t[
                                            :, pc0:pc0 + np_t
                                        ].unsqueeze(2).broadcast_to([P, np_t, P]),
                                        op=mybir.AluOpType.is_equal,
                                    )
                            if l == 4:
                                pa = ps_a.tile([P, P], f32, tag="pa")
                                for j, (qq, lc, pc) in enumerate(pieces):
                                    nc.tensor.matmul(
                                        out=pa[:64, :], lhsT=gbuf[:, lc, 0:64],
                                        rhs=St[:, (2 * j) * P:(2 * j + 1) * P],
                                        start=(j == 0), stop=False,
                                    )
                                    nc.tensor.matmul(
                                        out=pa[:64, :], lhsT=gbuf[:, lc, 64:128],
                                        rhs=St[:, (2 * j + 1) * P:(2 * j + 2) * P],
                                        start=False, stop=False,
                                    )
                                nc.tensor.matmul(
                                    out=pa[:64, :], lhsT=h_s[:, t * 64:(t + 1) * 64],
                                    rhs=ident_t[:], start=(np_t == 0), stop=True,
                                )
                                qT = qpool.tile([64, P], bf16, tag="qT")
                                nc.vector.tensor_copy(out=qT[:], in_=pa[:64, :])
                                defer.append((t, qT))
                                continue
                            pa_t = ps_a.tile([P, P], f32, tag="pa")
                            pa = pa_t[:, :AD]
                            if pair:
                                for j, (qq, lc, pc) in enumerate(pieces):
                                    nc.tensor.matmul(
                                        out=pa,
                                        lhsT=St[:, (2 * j) * P:(2 * j + 1) * P],
                                        rhs=gbuf[:, lc, 0:64],
                                        start=(j == 0), stop=False,
                                    )
                                    nc.tensor.matmul(
                                        out=pa,
                                        lhsT=St[:, (2 * j + 1) * P:(2 * j + 2) * P],
                                        rhs=gbuf[:, lc, 64:128],
                                        start=False, stop=False,
                                    )
                            else:
                                for j, (qq, lc, pc) in enumerate(pieces):
                                    nc.tensor.matmul(
                                        out=pa,
                                        lhsT=St[:, j * P:(j + 1) * P],
                                        rhs=gbuf[:, lc, :AD],
                                        start=(j == 0), stop=False,
                                    )
                            nc.tensor.matmul(
                                out=pa, lhsT=ident_t[:],
                                rhs=h_s[:, t * AD:(t + 1) * AD],
                                start=(np_t == 0), stop=False,
                            )
                            nc.tensor.matmul(
                                out=pa,
                                lhsT=rdinv_t[:, t * P:(t + 1) * P],
                                rhs=Bt[l][:], start=False, stop=True,
                            )
                            if l == 3:
                                # h5 = relu(pa*dinv)*dinv = relu(pa*dinv^2)
                                nc.scalar.activation(
                                    out=h_s[:, t * 64:(t + 1) * 64], in_=pa,
                                    func=mybir.ActivationFunctionType.Relu,
                                    scale=dinv2_t[:, t:t + 1],
                                )
                                nc.sync.dma_start(
                                    out=in_b[4][q].ap()[r0:r0 + P, :],
                                    in_=h_s[:, t * 64:(t + 1) * 64],
                                )
                            else:
                                act_t = epool.tile([P, AD], bf16, tag="actn")
                                nc.scalar.activation(
                                    out=act_t[:], in_=pa,
                                    func=mybir.ActivationFunctionType.Relu,
                                    scale=dinv_t[:, t:t + 1],
                                )
                                defer.append((t, act_t))
                        if l < 3:
                            pending.append(
                                lambda d=defer, l=l, g=g, a=actT, h=h_s:
                                flush_epi(l, g, d, a, h)
                            )
                        elif l == 4:
                            pending.append(lambda d=defer: flush_y(d))
                        # deferred AG triggers: emitted well after their input
                        # groups so the Pool-sequencer wait is pre-satisfied
                        if l < 4 and g in AGQ_AT:
                            emit_AG(l + 1, AGQ_AT[g])
                    flush()
                    if l < 4:
                        emit_AG(l + 1, 3)
    return nc


def kernel(**inputs):
    edge_index = np.asarray(inputs["edge_index"])
    key = edge_index.tobytes()[:64]
    if "prep" not in _cache or _cache.get("key") != key:
        _cache["key"] = key
        _cache["prep"] = _host_prep(edge_index)
        _cache.pop("runner", None)
    dinv, idx_row_w, idx_pair_w, dsel, dsel_par, meta = _cache["prep"]
    Ws, Bs = _fold_weights(inputs)
    bf = ml_dtypes.bfloat16

    x = np.asarray(inputs["x"], np.float32)
    xpad = np.zeros((NC, SH, IN), np.float32)
    xpad[:, :SR] = x.reshape(NC, SR, IN)
    dinvpad = np.ones((NC, SH), np.float32)
    dinvpad[:, :SR] = dinv.reshape(NC, SR)
    rdinvpad = 1.0 / dinvpad

    iotaw = np.tile(np.arange(P, dtype=np.float32)[None, :], (P, 1))

    if "runner" not in _cache:
        nc = _build_nc(meta)
        _cache["runner"] = _SpmdRunner(nc, NC)
    r = _cache["runner"]

    in_maps = []
    for c in range(NC):
        m = {
            "xT": np.ascontiguousarray(xpad[c].T).astype(bf),
            "dinv": np.ascontiguousarray(dinvpad[c].reshape(TP, P).T),
            "dinv2": np.ascontiguousarray((dinvpad[c] ** 2).reshape(TP, P).T),
            "rdinv": rdinvpad[c].reshape(1, SH).astype(bf),
            "idxr": idx_row_w[c],
            "idxp": idx_pair_w[c],
            "dsel": dsel[c].astype(bf),
            "dselp": dsel_par[c].astype(bf),
            "iotaw": iotaw.astype(bf),
            "ident": np.eye(P, dtype=np.float32).astype(bf),
        }
        for i in range(5):
            m[f"W{i+1}"] = Ws[i].astype(bf)
            m[f"B{i+1}"] = Bs[i].astype(bf)
        in_maps.append(m)

    r.put_inputs(in_maps)
    outs = r.run()
    res = r.results(outs)
    y = np.concatenate([res[c]["y"][:SR] for c in range(NC)], axis=0)[:N]
    return np.ascontiguousarray(y, dtype=np.float32)


# revision 24
# speedup vs baseline: 1.3711x; 1.3711x over previous
"""GCN (5-layer PyG GCNConv + BatchNorm eval + ReLU) on 8 Trainium2 NeuronCores.

v3 design. Nodes dst-sharded across 8 cores (12544 padded rows, 98 tiles of
128); edges follow their destination. The five layers run as one rolling
group-level software pipeline: for each group of dst tiles, edge-source rows
are gathered (dma_gather, 4 SWDGE queues round-robin), per-tile one-hot
selection matrices are built on DVE, aggregation runs as PSUM-accumulating
matmuls, the epilogue (fused BN+ReLU) feeds the NEXT layer's H-phase matmuls
for the same tiles, and chunked AllGathers of the next layer's node features
are triggered at quarter boundaries so collectives overlap the gather drain.

Gather descriptors are the bottleneck (~7.5ns/descriptor/queue), so edges are
packed into pooled per-(group,quarter) blocks (tile-sorted, boundary blocks
mix two dst tiles and get one masked one-hot matmul per tile) instead of
per-tile rounding: ~211k descriptors/layer vs 251k. Layers 4 and 5 use 64-wide
tables gathered as 2-node pairs (256B elements) with parity-split one-hots,
and layer 5 aggregates h4_act directly, applying W5 after aggregation (the
"H phase" of layer 5 is a single fused scale inside layer 4's epilogue).

GCN_REPS=n unrolls the computation n times on-device so test.py can measure
execution time as a wall-clock slope independent of tunnel latency.
"""
import os
import numpy as np
import ml_dtypes

N = 100000
E = 1600000
IN = 128
H = 128
C = 2
EPS = 1e-5
NC = 8
SR = 12500
P = 128
TP = 98
SH = TP * P
NQ = 4
QT = [25, 25, 24, 24]
QTOFF = [0, 25, 50, 74]
QROFF = [0, 3200, 6400, 9472]
QROWS = [3200, 3200, 3072, 3072]
GROUPS = [5] * 10 + [4] * 12
NG = len(GROUPS)
GOFF = np.cumsum([0] + GROUPS).tolist()
QG = [0] * 5 + [1] * 5 + [2] * 6 + [3] * 6
QLASTG = [4, 9, 15, 21]
QFIRSTG = [0, 5, 10, 16]
OC = [128, 128, 128, 64, 64]   # gather-table width per layer
DIMS = [(IN, H), (H, H), (H, H), (H, H // 2), (H // 2, C)]

_cache = {}

# ---------------------------------------------------------------------------
# Tile patch: walrus in this container rejects TPB_CTRL/extended instructions
# with >1 sync wait. Split waits across single-wait NOPs.
# ---------------------------------------------------------------------------


def _apply_tile_patch():
    if _cache.get("patched"):
        return
    _cache["patched"] = True
    import concourse.tile as tile_mod
    import concourse.mybir as mybir
    from concourse.vector_clock import ScopedClock

    MAXW = 1

    def _patched_drain_and_barrier(self, tick_clock, wait_clock):
        nc = self.nc
        probe = nc.sync.nop(nofuse=True)
        wait_clock.add_sem_waits(probe.ins, ScopedClock({None: tick_clock.global_clock}))
        si = probe.ins.sync_info
        if si is not None and si.on_wait and len(si.on_wait) > MAXW:
            waits = list(si.on_wait)
            si.on_wait = waits[:MAXW]
            for k in range(MAXW, len(waits), MAXW):
                extra = nc.sync.nop(nofuse=True)
                esi = extra.ins.sync_info
                if esi is None:
                    extra.ins.sync_info = mybir.SyncInfo(
                        on_wait=waits[k:k + MAXW], on_update=[]
                    )
                else:
                    esi.on_wait = waits[k:k + MAXW]
        nc.sync.drain()
        nc.all_engine_barrier()
        assert self.sems is not None
        popped = nc._tile_sem_poison_stack.pop()
        assert popped is self._sem_poison
        nc.clear_and_free_semaphores(list(self.sems.allocated().values()))
        nc.all_engine_barrier()

    tile_mod.TileContext._drain_and_barrier = _patched_drain_and_barrier

    _orig_commit = tile_mod.TileContext._commit_instruction

    def _patched_commit_instruction(self, inst, lazy_reg_writes=True):
        si = getattr(inst, "sync_info", None)
        if (
            si is not None
            and si.on_wait
            and len(si.on_wait) > MAXW
            and inst.engine != mybir.EngineType.Unassigned
        ):
            waits = list(si.on_wait)
            si.on_wait = waits[:MAXW]
            eng = self.nc.engines[inst.engine]
            for k in range(MAXW, len(waits), MAXW):
                extra = eng.nop(nofuse=True)
                esi = extra.ins.sync_info
                chunk = waits[k:k + MAXW]
                if esi is None:
                    extra.ins.sync_info = mybir.SyncInfo(on_wait=chunk, on_update=[])
                else:
                    esi.on_wait = chunk
        return _orig_commit(self, inst, lazy_reg_writes)

    tile_mod.TileContext._commit_instruction = _patched_commit_instruction


# ---------------------------------------------------------------------------
# SPMD runner: compile once via bass2jax/PJRT, keep the jitted fn for reuse.
# ---------------------------------------------------------------------------


class _SpmdRunner:
    def __init__(self, nc, n_cores=8):
        import jax
        from jax.sharding import Mesh, PartitionSpec, NamedSharding
        from jax.experimental.shard_map import shard_map
        import concourse.mybir as mybir
        from concourse.bass2jax import (
            _bass_exec_p,
            install_neuronx_cc_hook,
            partition_id_tensor,
        )
        from concourse.library_overlay import lower_extended_insts

        lower_extended_insts(nc)
        install_neuronx_cc_hook()
        self.jax = jax
        self.n_cores = n_cores
        partition_name = nc.partition_id_tensor.name if nc.partition_id_tensor else None
        in_names, out_names, out_avals, zero_outs = [], [], [], []
        for alloc in nc.m.functions[0].allocations:
            if not isinstance(alloc, mybir.MemoryLocationSet):
                continue
            name = alloc.memorylocations[0].name
            if alloc.kind == "ExternalInput":
                if name != partition_name:
                    in_names.append(name)
            elif alloc.kind == "ExternalOutput":
                out_names.append(name)
                shape = tuple(alloc.tensor_shape)
                dtype = mybir.dt.np(alloc.dtype)
                out_avals.append(jax.core.ShapedArray(shape, dtype))
                zero_outs.append(np.zeros(shape, dtype))
        self.in_names = list(in_names)
        self.out_names = out_names
        self.out_avals = out_avals
        self.zero_outs = zero_outs
        n_params = len(in_names)
        n_outs = len(out_avals)
        all_in_names = list(in_names) + list(out_names)
        if partition_name is not None:
            all_in_names.append(partition_name)

        def _body(*args):
            operands = list(args)
            if partition_name is not None:
                operands.append(partition_id_tensor())
            outs = _bass_exec_p.bind(
                *operands,
                out_avals=tuple(out_avals),
                in_names=tuple(all_in_names),
                out_names=tuple(out_names),
                lowering_input_output_aliases=(),
                sim_require_finite=True,
                sim_require_nnan=True,
                nc=nc,
            )
            return tuple(outs)

        devices = jax.devices()[:n_cores]
        self.mesh = Mesh(np.asarray(devices), ("core",))
        in_specs = (PartitionSpec("core"),) * (n_params + n_outs)
        out_specs = (PartitionSpec("core"),) * n_outs
        self.sharding = NamedSharding(self.mesh, PartitionSpec("core"))
        self.fn = jax.jit(
            shard_map(
                _body, mesh=self.mesh, in_specs=in_specs, out_specs=out_specs,
                check_rep=False,
            ),
            keep_unused=True,
        )
        self.n_params = n_params

    def put_inputs(self, in_maps):
        jax = self.jax
        per_core = [[np.asarray(m[name]) for name in self.in_names] for m in in_maps]
        concat_in = [
            np.concatenate([per_core[c][i] for c in range(self.n_cores)], axis=0)
            for i in range(self.n_params)
        ]
        self.dev_in = [jax.device_put(a, self.sharding) for a in concat_in]
        self.dev_zeros = [
            jax.device_put(
                np.zeros((self.n_cores * z.shape[0], *z.shape[1:]), z.dtype),
                self.sharding,
            )
            for z in self.zero_outs
        ]
        jax.block_until_ready(self.dev_in)

    def run(self):
        outs = self.fn(*self.dev_in, *self.dev_zeros)
        self.jax.block_until_ready(outs)
        return outs

    def results(self, outs):
        res = []
        for c in range(self.n_cores):
            res.append(
                {
                    name: np.asarray(outs[i]).reshape(
                        self.n_cores, *self.out_avals[i].shape
                    )[c]
                    for i, name in enumerate(self.out_names)
                }
            )
        return res


# ---------------------------------------------------------------------------
# Host-side graph partitioning
# ---------------------------------------------------------------------------


def _host_prep(edge_index):
    src = np.asarray(edge_index[0], dtype=np.int64)
    dst = np.asarray(edge_index[1], dtype=np.int64)
    deg = np.bincount(dst, minlength=N).astype(np.float32) + 1.0
    dinv = (1.0 / np.sqrt(deg)).astype(np.float32)

    core = dst // SR
    dl = dst - core * SR
    tile = dl // P
    slot = dl % P
    grp = np.searchsorted(np.asarray(GOFF[1:]), tile, side="right")

    cs = src // SR
    r = src - cs * SR
    q = (r >= QROFF[1]).astype(np.int64) + (r >= QROFF[2]) + (r >= QROFF[3])
    qrows = np.asarray(QROWS)[q]
    qidx = cs * qrows + (r - np.asarray(QROFF)[q])

    cell = (core * NG + grp) * NQ + q
    order = np.lexsort((qidx, tile, cell))
    cell_s = cell[order]
    tile_s = tile[order]
    slot_s = slot[order]
    qidx_s = qidx[order]

    ncells = NC * NG * NQ
    cnt = np.bincount(cell_s, minlength=ncells).reshape(NC, NG, NQ)
    blocks = np.maximum(np.ceil(cnt.max(axis=0) / P).astype(np.int64), 1)

    cumq = np.zeros((NG, NQ + 1), np.int64)
    for g in range(NG):
        cumq[g, 1:] = np.cumsum(blocks[g])
    gw = cumq[:, -1]
    goffc = np.concatenate([[0], np.cumsum(gw)])
    DW = int(goffc[-1])

    cstart = np.zeros(ncells + 1, np.int64)
    np.cumsum(cnt.reshape(-1), out=cstart[1:])
    rank = np.arange(E) - cstart[cell_s]
    g_s = (cell_s // NQ) % NG
    q_s = cell_s % NQ
    c_s = cell_s // (NG * NQ)
    colrel = rank // P
    rowrel = rank % P
    gcol = goffc[g_s] + cumq[g_s, q_s] + colrel

    # pieces keyed/sorted by (group, tile, quarter, block) so each tile's
    # dsel columns are contiguous
    KQ = int(blocks.max()) + 1
    key = ((tile_s * NQ + q_s) * KQ) + colrel
    uk = np.unique(key)
    NPIECE = len(uk)
    pcol = np.searchsorted(uk, key)
    piece_tile = uk // (NQ * KQ)
    piece_q = (uk // KQ) % NQ
    piece_lc = uk % KQ  # block within (g, q)

    dsel = np.full((NC, P, NPIECE), -1.0, np.float32)
    dsel[c_s, rowrel, pcol] = slot_s.astype(np.float32)
    dsel_par = np.full((NC, P, 2 * NPIECE), -1.0, np.float32)
    dsel_par[c_s, rowrel, 2 * pcol + (qidx_s & 1)] = slot_s.astype(np.float32)

    idx_row = np.zeros((NC, DW, P), np.int16)
    idx_row[c_s, gcol, rowrel] = qidx_s.astype(np.int16)
    idx_pair = np.zeros((NC, DW, P), np.int16)
    idx_pair[c_s, gcol, rowrel] = (qidx_s >> 1).astype(np.int16)

    def wrap(idx_flat):
        iw = np.zeros((NC, P, DW * 8), np.int16)
        for g in range(NG):
            for qq in range(NQ):
                c0 = int(goffc[g] + cumq[g, qq])
                w = int(blocks[g, qq])
                seq = idx_flat[:, c0:c0 + w, :].reshape(NC, w * P)
                w16 = seq.reshape(NC, w * P // 16, 16).transpose(0, 2, 1)
                iw[:, :, c0 * 8:(c0 + w) * 8] = np.tile(w16, (1, 8, 1))
        return iw

    idx_row_w = wrap(idx_row)
    idx_pair_w = wrap(idx_pair)

    # per-(group, tile) piece lists: [(q, local block col in group, piece col)]
    tg = np.searchsorted(np.asarray(GOFF[1:]), piece_tile, side="right")
    plan = [dict() for _ in range(NG)]
    for i in range(NPIECE):
        g = int(tg[i])
        t = int(piece_tile[i])
        lc = int(cumq[g, piece_q[i]] + piece_lc[i])
        plan[g].setdefault(t, []).append((int(piece_q[i]), lc, i))

    meta = dict(blocks=blocks.tolist(), cumq=cumq.tolist(),
                gw=gw.tolist(), goffc=goffc.tolist(), DW=DW,
                NPIECE=NPIECE, plan=plan)
    return dinv, idx_row_w, idx_pair_w, dsel, dsel_par, meta


def _fold_weights(inputs):
    Ws, Bs = [], []
    for i in range(1, 6):
        W = np.asarray(inputs[f"W{i}"], np.float32)
        b = np.asarray(inputs[f"b{i}"], np.float32)
        if i <= 4:
            g = np.asarray(inputs[f"g{i}"], np.float32)
            be = np.asarray(inputs[f"be{i}"], np.float32)
            rm = np.asarray(inputs[f"rm{i}"], np.float32)
            rv = np.asarray(inputs[f"rv{i}"], np.float32)
            s = g / np.sqrt(rv + EPS)
            W = W * s[None, :]
            b = b * s + be - rm * s
        Ws.append(np.ascontiguousarray(W, dtype=np.float32))
        Bs.append(b.astype(np.float32)[None, :])
    return Ws, Bs


# ---------------------------------------------------------------------------
# Device program
# ---------------------------------------------------------------------------


def _build_nc(meta):
    REPS = int(os.environ.get("GCN_REPS", 1))
    NQUEUES = int(os.environ.get("GCN_QUEUES", 4))
    import concourse.bass as bass
    import concourse.mybir as mybir
    from concourse.tile import TileContext
    from concourse import library_config

    _apply_tile_patch()

    f32 = mybir.dt.float32
    bf16 = mybir.dt.bfloat16
    i16 = mybir.dt.int16
    nc = bass.Bass(
        "TRN2", target_bir_lowering=False, debug=False, num_swdge_queues=NQUEUES
    )

    blocks = meta["blocks"]      # [NG][NQ]
    cumq = meta["cumq"]          # [NG][NQ+1]
    gw = meta["gw"]              # [NG]
    goffc = meta["goffc"]        # [NG+1]
    DW = meta["DW"]
    NPIECE = meta["NPIECE"]
    plan = meta["plan"]          # [NG] {tile: [(q, lc, piece)]}
    GWMAX = max(gw)
    NPT_MAX = max(len(v) for g in range(NG) for v in plan[g].values())
    BMAX = max(max(b) for b in blocks)

    xT_in = nc.declare_dram_parameter("xT", [IN, SH], bf16, isOutput=False)
    dinv_in = nc.declare_dram_parameter("dinv", [P, TP], f32, isOutput=False)
    dinv2_in = nc.declare_dram_parameter("dinv2", [P, TP], f32, isOutput=False)
    rdinv_in = nc.declare_dram_parameter("rdinv", [1, SH], bf16, isOutput=False)
    idxr_in = nc.declare_dram_parameter("idxr", [P, DW * 8], i16, isOutput=False)
    idxp_in = nc.declare_dram_parameter("idxp", [P, DW * 8], i16, isOutput=False)
    dsel_in = nc.declare_dram_parameter("dsel", [P, NPIECE], bf16, isOutput=False)
    dselp_in = nc.declare_dram_parameter("dselp", [P, 2 * NPIECE], bf16, isOutput=False)
    iota_in = nc.declare_dram_parameter("iotaw", [P, P], bf16, isOutput=False)
    ident_in = nc.declare_dram_parameter("ident", [P, P], bf16, isOutput=False)
    W_in = [nc.declare_dram_parameter(f"W{i+1}", list(DIMS[i]), bf16, isOutput=False)
            for i in range(5)]
    B_in = [nc.declare_dram_parameter(f"B{i+1}", [1, DIMS[i][1]], bf16, isOutput=False)
            for i in range(5)]
    y_out = nc.declare_dram_parameter("y", [SH, C], f32, isOutput=True)

    in_b = [
        [nc.dram_tensor(f"in_b{l}_{q}", [QROWS[q], OC[l]], bf16) for q in range(NQ)]
        for l in range(5)
    ]
    hs = []
    for l in range(5):
        row = []
        for q in range(NQ):
            if OC[l] == 128:
                shp = [NC * QROWS[q], 128]
            else:
                shp = [NC * QROWS[q] // 2, 128]
            row.append(nc.dram_tensor(f"hs{l}_{q}", shp, bf16, addr_space="Shared"))
        hs.append(row)

    with TileContext(nc) as tc:
        with (
            tc.tile_pool(name="const", bufs=1) as cpool,
            tc.tile_pool(name="act", bufs=1) as apool,
            tc.tile_pool(name="hbuf", bufs=1) as hpool,
            tc.tile_pool(name="gath", bufs=int(os.environ.get("GCN_GBUFS", 2))) as gpool,
            tc.tile_pool(name="sel", bufs=3) as sbpool,
            tc.tile_pool(name="idxs", bufs=8) as ipool,
            tc.tile_pool(name="work", bufs=4) as wpool,
            tc.tile_pool(name="epi", bufs=8) as epool,
            tc.tile_pool(name="qp", bufs=8) as qpool,
            tc.tile_pool(name="ps_h", bufs=2, space="PSUM") as ps_h,
            tc.tile_pool(name="ps_a", bufs=2, space="PSUM") as ps_a,
            tc.tile_pool(name="ps_t", bufs=2, space="PSUM") as ps_t,
            tc.tile_pool(name="ps_y", bufs=2, space="PSUM") as ps_y,
        ):
            nc.gpsimd.load_library(library_config.mlp)
            nid_vals = sorted({blocks[g][q] * P for g in range(NG) for q in range(NQ)})
            nid_regs = {}
            for v in nid_vals:
                reg = nc.alloc_register(mybir.EngineType.Pool, f"nid{v}")
                nc.gpsimd.reg_mov(reg, v)
                nid_regs[v] = reg

            Wt, Bt = [], []
            for l in range(5):
                w = cpool.tile(list(DIMS[l]), bf16, name=f"Wt{l}")
                nc.sync.dma_start(out=w[:], in_=W_in[l][:])
                Wt.append(w)
                b = cpool.tile([1, DIMS[l][1]], bf16, name=f"Bt{l}")
                nc.sync.dma_start(out=b[:], in_=B_in[l][:])
                Bt.append(b)
            iota_t = cpool.tile([P, P], bf16)
            nc.sync.dma_start(out=iota_t[:], in_=iota_in[:])
            ident_t = cpool.tile([P, P], bf16)
            nc.sync.dma_start(out=ident_t[:], in_=ident_in[:])
            dinv_t = cpool.tile([P, TP], f32)
            nc.sync.dma_start(out=dinv_t[:], in_=dinv_in[:])
            dinv2_t = cpool.tile([P, TP], f32)
            nc.sync.dma_start(out=dinv2_t[:], in_=dinv2_in[:])
            rdinv_t = cpool.tile([1, SH], bf16)
            nc.sync.dma_start(out=rdinv_t[:], in_=rdinv_in[:])
            dsel_t = cpool.tile([P, NPIECE], bf16)
            nc.scalar.dma_start(out=dsel_t[:], in_=dsel_in[:])
            dselp_t = cpool.tile([P, 2 * NPIECE], bf16)
            nc.scalar.dma_start(out=dselp_t[:], in_=dselp_in[:])

            idxr_t = cpool.tile([P, DW * 8], i16)
            nc.scalar.dma_start(out=idxr_t[:], in_=idxr_in[:])

            qcount = [0]
            AGQ_AT = {11: 0, 16: 1, 20: 2}
            pending = []

            def flush():
                for fn in pending:
                    fn()
                pending.clear()

            def emit_H(l, g, actT, h_s):
                # layer l's H phase for group g's tiles (l = 0..3)
                I, O = DIMS[l]
                for t in range(GOFF[g], GOFF[g + 1]):
                    q = QG[g]
                    ps = ps_h.tile([P, O], f32, tag="ps_h")
                    nc.tensor.matmul(
                        out=ps[:], lhsT=actT[:I, t * P:(t + 1) * P],
                        rhs=Wt[l][:], start=True, stop=True,
                    )
                    nc.vector.tensor_scalar_mul(
                        out=h_s[:, t * O:(t + 1) * O], in0=ps[:],
                        scalar1=dinv_t[:, t:t + 1],
                    )
                    r0 = t * P - QROFF[q]
                    nc.sync.dma_start(
                        out=in_b[l][q].ap()[r0:r0 + P, :],
                        in_=h_s[:, t * O:(t + 1) * O],
                    )

            def emit_AG(l, q):
                if os.environ.get("GCN_NOCC"):
                    nrows = QROWS[q] if OC[l] == 128 else QROWS[q] // 2
                    nc.sync.dma_start(
                        out=hs[l][q].ap()[:nrows, :], in_=in_b[l][q][:]
                    )
                    return
                nc.gpsimd.collective_compute(
                    "AllGather",
                    mybir.AluOpType.bypass,
                    ins=[in_b[l][q][:]],
                    outs=[hs[l][q][:]],
                    replica_groups=[list(range(NC))],
                )

            def flush_epi(l, g, defer, actT, h_s):
                # deferred epilogue (PE transpose + DVE copy) and next-layer H
                for t, act_t in defer:
                    pt = ps_t.tile([P, P], bf16, tag="pt")
                    nc.tensor.transpose(
                        out=pt[:128, :], in_=act_t[:], identity=ident_t[:]
                    )
                    nc.vector.tensor_copy(
                        out=actT[:128, t * P:(t + 1) * P], in_=pt[:128, :]
                    )
                emit_H(l + 1, g, actT, h_s)

            def flush_y(defer):
                # deferred layer-5 output: W5 matmul + bias + dinv scale + DMA
                for t, qT in defer:
                    yp = ps_y.tile([P, C], f32, tag="yp")
                    nc.tensor.matmul(
                        out=yp[:], lhsT=qT[:], rhs=Wt[4][:],
                        start=True, stop=False,
                    )
                    nc.tensor.matmul(
                        out=yp[:], lhsT=rdinv_t[:, t * P:(t + 1) * P],
                        rhs=Bt[4][:], start=False, stop=True,
                    )
                    yt = wpool.tile([P, C], f32, tag="yt")
                    nc.vector.tensor_scalar_mul(
                        out=yt[:], in0=yp[:], scalar1=dinv_t[:, t:t + 1]
                    )
                    nc.sync.dma_start(
                        out=y_out.ap()[t * P:(t + 1) * P, :], in_=yt[:]
                    )

            for rep in range(REPS):
                actT = apool.tile([P, SH], bf16, tag="actT")
                nc.sync.dma_start(out=actT[:IN, :], in_=xT_in[:])
                h_s = hpool.tile([P, TP * P], bf16, tag="h_s")

                # prologue: layer 0 H phase + AllGathers
                for g in range(NG):
                    emit_H(0, g, actT, h_s)
                    if g in QLASTG:
                        emit_AG(0, QG[g])

                for l in range(int(os.environ.get("GCN_LAYERS", 5))):
                    O = OC[l]          # gather table width
                    pair = O == 64
                    AD = 64 if l == 4 else DIMS[l][1]   # agg feature width

                    for g in range(NG):
                        flush()
                        # --- gathers for (g, q) ---
                        gbuf = gpool.tile([P, GWMAX, P], bf16, tag="g")
                        for q in range(NQ):
                            w = blocks[g][q]
                            c0 = goffc[g] + cumq[g][q]
                            lc0 = cumq[g][q]
                            if os.environ.get("GCN_NOGATHER"):
                                nc.sync.dma_start(
                                    out=gbuf[:, lc0:lc0 + w, :],
                                    in_=hs[l][q].ap()[:w * P, :],
                                )
                                qcount[0] += 1
                                continue
                            if pair:
                                it = ipool.tile([P, BMAX * 8], i16, tag="idx")
                                nc.scalar.dma_start(
                                    out=it[:, :w * 8],
                                    in_=idxp_in.ap()[:, c0 * 8:(c0 + w) * 8],
                                )
                                idx_ap = it[:, :w * 8]
                            else:
                                idx_ap = idxr_t[:, c0 * 8:(c0 + w) * 8]
                            nc.gpsimd.dma_gather(
                                out_ap=gbuf[:, lc0:lc0 + w, :],
                                in_ap=hs[l][q].ap()[:, :],
                                idxs_ap=idx_ap,
                                num_idxs=w * P,
                                num_idxs_reg=nid_regs[w * P],
                                elem_size=P,
                                single_packet=bool(os.environ.get("GCN_SP")),
                                # rotate the quarter->queue mapping per group
                                # so no queue permanently carries the larger
                                # quarters' descriptor load
                                queue_num=(qcount[0] + qcount[0] // NQ)
                                % NQUEUES,
                            )
                            qcount[0] += 1
                        # --- per-tile: S build, agg matmuls, epilogue ---
                        defer = []
                        for t in range(GOFF[g], GOFF[g + 1]):
                            pieces = plan[g].get(t, [])
                            if os.environ.get("GCN_MMSKIP"):
                                pieces = []
                            np_t = len(pieces)
                            q = QG[g]
                            r0 = t * P - QROFF[q]
                            if np_t:
                                pc0 = pieces[0][2]
                                St = sbpool.tile([P, 2 * NPT_MAX * P], bf16, tag="S")
                                if os.environ.get("GCN_NOSBUILD"):
                                    pass
                                elif pair:
                                    nc.vector.tensor_tensor(
                                        out=St[:, :2 * np_t * P].rearrange(
                                            "p (b q) -> p b q", b=2 * np_t
                                        ),
                                        in0=iota_t[:].unsqueeze(1).broadcast_to(
                                            [P, 2 * np_t, P]
                                        ),
                                        in1=dselp_t[
                                            :, 2 * pc0:2 * (pc0 + np_t)
                                        ].unsqueeze(2).broadcast_to([P, 2 * np_t, P]),
                                        op=mybir.AluOpType.is_equal,
                                    )
                                else:
                                    nc.vector.tensor_tensor(
                                        out=St[:, :np_t * P].rearrange(
                                            "p (b q) -> p b q", b=np_t
                                        ),
                                        in0=iota_t[:].unsqueeze(1).broadcast_to(
                                            [P, np_t, P]
                                        ),
                                        in1=dsel_t[
                                            :, pc0:pc0 + np_t
                                        ].unsqueeze(2).broadcast_to([P, np_t, P]),
                                        op=mybir.AluOpType.is_equal,
                                    )
                            if l == 4:
                                pa = ps_a.tile([P, P], f32, tag="pa")
                                for j, (qq, lc, pc) in enumerate(pieces):
                                    nc.tensor.matmul(
                                        out=pa[:64, :], lhsT=gbuf[:, lc, 0:64],
                                        rhs=St[:, (2 * j) * P:(2 * j + 1) * P],
                                        start=(j == 0), stop=False,
                                    )
                                    nc.tensor.matmul(
                                        out=pa[:64, :], lhsT=gbuf[:, lc, 64:128],
                                        rhs=St[:, (2 * j + 1) * P:(2 * j + 2) * P],
                                        start=False, stop=False,
                                    )
                                nc.tensor.matmul(
                                    out=pa[:64, :], lhsT=h_s[:, t * 64:(t + 1) * 64],
                                    rhs=ident_t[:], start=(np_t == 0), stop=True,
                                )
                                qT = qpool.tile([64, P], bf16, tag="qT")
                                nc.vector.tensor_copy(out=qT[:], in_=pa[:64, :])
                                defer.append((t, qT))
                                continue
                            pa_t = ps_a.tile([P, P], f32, tag="pa")
                            pa = pa_t[:, :AD]
                            if pair:
                                for j, (qq, lc, pc) in enumerate(pieces):
                                    nc.tensor.matmul(
                                        out=pa,
                                        lhsT=St[:, (2 * j) * P:(2 * j + 1) * P],
                                        rhs=gbuf[:, lc, 0:64],
                                        start=(j == 0), stop=False,
                                    )
                                    nc.tensor.matmul(
                                        out=pa,
                                        lhsT=St[:, (2 * j + 1) * P:(2 * j + 2) * P],
                                        rhs=gbuf[:, lc, 64:128],
                                        start=False, stop=False,
                                    )
                            else:
                                for j, (qq, lc, pc) in enumerate(pieces):
                                    nc.tensor.matmul(
                                        out=pa,
                                        lhsT=St[:, j * P:(j + 1) * P],
                                        rhs=gbuf[:, lc, :AD],
                                        start=(j == 0), stop=False,
                                    )
                            nc.tensor.matmul(
                                out=pa, lhsT=ident_t[:],
                                rhs=h_s[:, t * AD:(t + 1) * AD],
                                start=(np_t == 0), stop=False,
                            )
                            nc.tensor.matmul(
                                out=pa,
                                lhsT=rdinv_t[:, t * P:(t + 1) * P],
                                rhs=Bt[l][:], start=False, stop=True,
                            )
                            if l == 3:
                                # h5 = relu(pa*dinv)*dinv = relu(pa*dinv^2)
                                nc.scalar.activation(
                                    out=h_s[:, t * 64:(t + 1) * 64], in_=pa,
                                    func=mybir.ActivationFunctionType.Relu,
                                    scale=dinv2_t[:, t:t + 1],
                                )
                                nc.sync.dma_start(
                                    out=in_b[4][q].ap()[r0:r0 + P, :],
                                    in_=h_s[:, t * 64:(t + 1) * 64],
                                )
                            else:
                                act_t = epool.tile([P, AD], bf16, tag="actn")
                                nc.scalar.activation(
                                    out=act_t[:], in_=pa,
                                    func=mybir.ActivationFunctionType.Relu,
                                    scale=dinv_t[:, t:t + 1],
                                )
                                defer.append((t, act_t))
                        if l < 3:
                            pending.append(
                                lambda d=defer, l=l, g=g, a=actT, h=h_s:
                                flush_epi(l, g, d, a, h)
                            )
                        elif l == 4:
                            pending.append(lambda d=defer: flush_y(d))
                        # deferred AG triggers: emitted well after their input
                        # groups so the Pool-sequencer wait is pre-satisfied
                        if l < 4 and g in AGQ_AT:
                            emit_AG(l + 1, AGQ_AT[g])
                    flush()
                    if l < 4:
                        emit_AG(l + 1, 3)
    return nc


def kernel(**inputs):
    edge_index = np.asarray(inputs["edge_index"])
    key = edge_index.tobytes()[:64]
    if "prep" not in _cache or _cache.get("key") != key:
        _cache["key"] = key
        _cache["prep"] = _host_prep(edge_index)
        _cache.pop("runner", None)
    dinv, idx_row_w, idx_pair_w, dsel, dsel_par, meta = _cache["prep"]
    Ws, Bs = _fold_weights(inputs)
    bf = ml_dtypes.bfloat16

    x = np.asarray(inputs["x"], np.float32)
    xpad = np.zeros((NC, SH, IN), np.float32)
    xpad[:, :SR] = x.reshape(NC, SR, IN)
    dinvpad = np.ones((NC, SH), np.float32)
    dinvpad[:, :SR] = dinv.reshape(NC, SR)
    rdinvpad = 1.0 / dinvpad

    iotaw = np.tile(np.arange(P, dtype=np.float32)[None, :], (P, 1))

    if "runner" not in _cache:
        nc = _build_nc(meta)
        _cache["runner"] = _SpmdRunner(nc, NC)
    r = _cache["runner"]

    in_maps = []
    for c in range(NC):
        m = {
            "xT": np.ascontiguousarray(xpad[c].T).astype(bf),
            "dinv": np.ascontiguousarray(dinvpad[c].reshape(TP, P).T),
            "dinv2": np.ascontiguousarray((dinvpad[c] ** 2).reshape(TP, P).T),
            "rdinv": rdinvpad[c].reshape(1, SH).astype(bf),
            "idxr": idx_row_w[c],
            "idxp": idx_pair_w[c],
            "dsel": dsel[c].astype(bf),
            "dselp": dsel_par[c].astype(bf),
            "iotaw": iotaw.astype(bf),
            "ident": np.eye(P, dtype=np.float32).astype(bf),
        }
        for i in range(5):
            m[f"W{i+1}"] = Ws[i].astype(bf)
            m[f"B{i+1}"] = Bs[i].astype(bf)
        in_maps.append(m)

    r.put_inputs(in_maps)
    outs = r.run()
    res = r.results(outs)
    y = np.concatenate([res[c]["y"][:SR] for c in range(NC)], axis=0)[:N]
    return np.ascontiguousarray(y, dtype=np.float32)
